# revision 25
# baseline (speedup 1.0000x reference)
"""Trainium2 Bass kernel for nn_AdditiveAttention (B=8, S=4096, D=1024, H=16).

Sharding: pure data-parallel over batch — 8 NeuronCores, one batch element
per core, weights replicated. No collectives.

Per-core layout: everything transposed (d on partitions, s on free).
  - Host feeds xt = X[b].T as bf16 [1024, 4096]; device returns out.T f32.
  - Q/K/V: out_t[d', s] = sum_d W[d, d'] * xt[d, s]  (lhsT = W tile, rhs = xt)
  - per-head logits via block-diag "replicated" weight matmul: stationary
    [128, 128] where column j holds head(j//64)'s logit weights — every
    partition of the PSUM output carries its own head's logit row.
  - softmax over s without max-subtraction (|logits| << 1 by construction),
    exp + denominator fused on ScalarE via accum_out.
  - pooled vectors gq/gk via tensor_tensor mult + reduce_sum on VectorE.
  - p = (K + bk) * gq and u = (V + bv) * gk fused into PSUM evacuation
    (per-partition tensor_scalar).
  - residual: q + bq written straight to DRAM during Q evacuation;
    (u @ Wr + br) DMA-accumulated on top at the end.

Loop order inside each big matmul phase is (m, n, k): each PSUM chunk
accumulates its full contraction consecutively, so only ~2 banks are live
at a time and the logit/rt matmuls never starve for PSUM.
"""

import sys
import types

import numpy as np
import ml_dtypes

from contextlib import ExitStack

import concourse.bass as bass
import concourse.tile as tile
from concourse import bacc, mybir
from concourse.bass_utils import run_bass_kernel_spmd

B, S, D, H, HD = 8, 4096, 1024, 16, 64
P = 128          # partitions
T = D // P       # 8 d-tiles
NC_ = 512        # psum chunk free size
NS = S // NC_    # 8 s-chunks
N_CORES = 8
BF16 = mybir.dt.bfloat16
FP8 = mybir.dt.float8e4
F32 = mybir.dt.float32
W8SCALE = 64.0   # host scales Wk/Wv by this into e4m3 normal range
BF = ml_dtypes.bfloat16
F8 = ml_dtypes.float8_e4m3

_CACHE = {}


def _build():
    nc = bacc.Bacc(
        "TRN2", target_bir_lowering=False, debug=False, num_devices=N_CORES
    )
    xt_ext = nc.declare_dram_parameter("xt", [D, S], BF16, isOutput=False)
    wq_ext = nc.declare_dram_parameter("wq", [D, D], BF16, isOutput=False)
    xt8_ext = nc.declare_dram_parameter("xt8", [D, S], FP8, isOutput=False)
    wk_ext = nc.declare_dram_parameter("wk", [D, D], FP8, isOutput=False)
    wv_ext = nc.declare_dram_parameter("wv", [D, D], FP8, isOutput=False)
    bq_ext = nc.declare_dram_parameter("bq", [P, T], F32, isOutput=False)
    bk_ext = nc.declare_dram_parameter("bk", [P, T], F32, isOutput=False)
    bv_ext = nc.declare_dram_parameter("bv", [P, T], F32, isOutput=False)
    wql_ext = nc.declare_dram_parameter("wqlrep", [P, P], BF16, isOutput=False)
    wkl_ext = nc.declare_dram_parameter("wklrep", [P, P], BF16, isOutput=False)
    wrr_ext = nc.declare_dram_parameter("wrr", [P, P], BF16, isOutput=False)
    br_ext = nc.declare_dram_parameter("br", [P, 1], F32, isOutput=False)
    out_ext = nc.declare_dram_parameter("out", [D, S], F32, isOutput=True)

    AX = mybir.AxisListType.X
    ALU = mybir.AluOpType
    AF = mybir.ActivationFunctionType

    with tile.TileContext(nc) as tc, ExitStack() as ctx:
        singles = ctx.enter_context(tc.tile_pool(name="singles", bufs=1))
        wpool = ctx.enter_context(tc.tile_pool(name="wpool", bufs=2))
        psum = ctx.enter_context(tc.tile_pool(name="psum", bufs=4, space="PSUM"))
        qt_pool = ctx.enter_context(tc.tile_pool(name="qt", bufs=2))
        e_pool = ctx.enter_context(tc.tile_pool(name="epool", bufs=1))
        p_pool = ctx.enter_context(tc.tile_pool(name="ppool", bufs=3))
        u_pool = ctx.enter_context(tc.tile_pool(name="upool", bufs=2))
        stage_pool = ctx.enter_context(tc.tile_pool(name="stage", bufs=6))
        eff_pool = ctx.enter_context(tc.tile_pool(name="eff", bufs=2))
        small_pool = ctx.enter_context(tc.tile_pool(name="small", bufs=4))

        # ---- resident inputs ----
        # xt loaded s-major (one 3D-AP DMA per s-chunk: all 8 k-tiles) so the
        # first (m0, n0) contraction can start after ~1 MB instead of 8.4 MB,
        # and only 8 DMA issues are paid on the sync engine.
        # 16 DMAs (k-tile x s-half) with 4 KB-contiguous rows, spread over two
        # issuing engines so all 16 DMA engines pull concurrently.
        xt_sb = singles.tile([P, T, S], BF16, name="xt", tag="xt")
        SQ = S // 4
        for q in range(4):
            qsl = slice(q * SQ, (q + 1) * SQ)
            for k in range(T):
                rsl = slice(k * P, (k + 1) * P)
                eng = nc.sync if k % 2 == 0 else nc.gpsimd
                eng.dma_start(xt_sb[:, k, qsl], xt_ext.ap()[rsl, qsl])
        wq_sb = wpool.tile([P, T, D], BF16, name="wmat", tag="wmat")
        for k in range(T):
            nc.scalar.dma_start(
                wq_sb[:, k, :], wq_ext.ap()[k * P : (k + 1) * P, :]
            )

        xt8_sb = singles.tile([P, T, S], FP8, name="xt8", tag="xt8")
        for k in range(T):
            nc.scalar.dma_start(
                xt8_sb[:, k, :], xt8_ext.ap()[k * P : (k + 1) * P, :]
            )

        def load_w(ext, dt):
            w_sb = wpool.tile([P, T, D], dt, name="wmat", tag="wmat")
            for k in range(T):
                nc.sync.dma_start(w_sb[:, k, :], ext.ap()[k * P : (k + 1) * P, :])
            return w_sb

        wqlrep = singles.tile([P, P], BF16, name="wqlrep", tag="wqlrep")
        nc.sync.dma_start(wqlrep[:], wql_ext.ap())
        wklrep = singles.tile([P, P], BF16, name="wklrep", tag="wklrep")
        nc.sync.dma_start(wklrep[:], wkl_ext.ap())
        wrr = singles.tile([P, P], BF16, name="wrr", tag="wrr")
        nc.sync.dma_start(wrr[:], wrr_ext.ap())
        bq_sb = singles.tile([P, T], F32, name="bq", tag="bq")
        nc.sync.dma_start(bq_sb[:], bq_ext.ap())
        bk_sb = singles.tile([P, T], F32, name="bk", tag="bk")
        nc.sync.dma_start(bk_sb[:], bk_ext.ap())
        bv_sb = singles.tile([P, T], F32, name="bv", tag="bv")
        nc.sync.dma_start(bv_sb[:], bv_ext.ap())
        br_sb = singles.tile([P, 1], F32, name="br", tag="br")
        nc.sync.dma_start(br_sb[:], br_ext.ap())

        gq_all = singles.tile([P, T], F32, name="gq", tag="gq")
        gk_all = singles.tile([P, T], F32, name="gk", tag="gk")

        def big_mm_chunk(w_sb, t, n):
            """One fully-accumulated [P, NC_] psum chunk of output tile t."""
            pch = psum.tile([P, NC_], F32, name="pch", tag="pch")
            for k in range(T):
                nc.tensor.matmul(
                    pch,
                    w_sb[:, k, t * P : (t + 1) * P],
                    xt_sb[:, k, n * NC_ : (n + 1) * NC_],
                    start=(k == 0),
                    stop=(k == T - 1),
                )
            return pch

        def big_mm_chunk_f8(w_sb, t, n):
            """fp8 DoubleRow version: k-tiles paired, 2 MACs/cell/cycle."""
            pch = psum.tile([P, NC_], F32, name="pch", tag="pch")
            for k in range(0, T, 2):
                nc.tensor.matmul(
                    pch,
                    w_sb[:, k : k + 2, t * P : (t + 1) * P],
                    xt8_sb[:, k : k + 2, n * NC_ : (n + 1) * NC_],
                    start=(k == 0),
                    stop=(k == T - 2),
                    perf_mode=mybir.MatmulPerfMode.DoubleRow,
                )
            return pch

        def pool_step(t, src_tile, wrep, g_all):
            """softmax(logits)-weighted sum of src_tile over s -> g_all[:, t]."""
            e = e_pool.tile([P, S], BF16, name="e", tag="e")
            sums = small_pool.tile([P, NS], F32, name="sums", tag="sums")
            for n in range(NS):
                pc = psum.tile([P, NC_], F32, name="plg", tag="plg")
                nc.tensor.matmul(
                    pc, wrep, src_tile[:, n * NC_ : (n + 1) * NC_],
                    start=True, stop=True,
                )
                nc.scalar.activation(
                    e[:, n * NC_ : (n + 1) * NC_], pc, AF.Exp,
                    bias=0.0, scale=1.0, accum_out=sums[:, n : n + 1],
                )
            stot = small_pool.tile([P, 1], F32, name="stot", tag="stot")
            nc.vector.reduce_sum(stot, sums, axis=AX)
            # fold the 1/W8SCALE un-scaling of the fp8 K/V matmuls into the
            # softmax normalizer: g_all ends up holding g / W8SCALE.
            nc.vector.tensor_scalar_mul(stot, stot, W8SCALE)
            recip = small_pool.tile([P, 1], F32, name="recip", tag="recip")
            nc.vector.reciprocal(recip, stot)
            graw = small_pool.tile([P, 1], F32, name="graw", tag="graw")
            nc.vector.tensor_tensor(e[:], e[:], src_tile[:], ALU.mult)
            nc.vector.reduce_sum(graw, e[:], axis=AX)
            nc.vector.tensor_mul(g_all[:, t : t + 1], graw, recip)

        # ---- Q phase ----
        for t in range(T):
            qt = qt_pool.tile([P, S], BF16, name="qt", tag="qt")
            for n in range(NS):
                sl = slice(n * NC_, (n + 1) * NC_)
                pch = big_mm_chunk(wq_sb, t, n)
                stg = stage_pool.tile([P, NC_], F32, name="stage", tag="stage")
                nc.scalar.activation(
                    stg, pch, AF.Identity, bias=bq_sb[:, t : t + 1], scale=1.0
                )
                nc.sync.dma_start(out_ext.ap()[t * P : (t + 1) * P, sl], stg)
                nc.vector.tensor_copy(qt[:, sl], stg)
            pool_step(t, qt, wqlrep, gq_all)

        # ---- K phase ----
        wk_sb = load_w(wk_ext, FP8)
        for t in range(T):
            # pk = 64*(k + bk), ungated: the gq gate (and the 1/64) ride in
            # the kl stationary and the gk finalizer instead.
            p = p_pool.tile([P, S], BF16, name="p", tag="p")
            for n in range(NS):
                pch = big_mm_chunk_f8(wk_sb, t, n)
                nc.scalar.activation(
                    p[:, n * NC_ : (n + 1) * NC_], pch, AF.Identity,
                    bias=bk_sb[:, t : t + 1], scale=1.0,
                )
            eff_kl = eff_pool.tile([P, P], BF16, name="effkl", tag="effkl")
            nc.vector.tensor_scalar_mul(eff_kl[:], wklrep[:], gq_all[:, t : t + 1])
            pool_step(t, p, eff_kl, gk_all)
            nc.vector.tensor_mul(
                gk_all[:, t : t + 1], gk_all[:, t : t + 1], gq_all[:, t : t + 1]
            )

        # ---- V phase + output (chunk-granular: u, rt, evac, DMA per n) ----
        wv_sb = load_w(wv_ext, FP8)
        for t in range(T):
            u = u_pool.tile([P, S], BF16, name="u", tag="u")
            eff_rt = eff_pool.tile([P, P], BF16, name="effrt", tag="effrt")
            nc.vector.tensor_scalar_mul(eff_rt[:], wrr[:], gk_all[:, t : t + 1])
            for n in range(NS):
                sl = slice(n * NC_, (n + 1) * NC_)
                pch = big_mm_chunk_f8(wv_sb, t, n)
                nc.scalar.activation(
                    u[:, sl], pch, AF.Identity,
                    bias=bv_sb[:, t : t + 1], scale=1.0,
                )
                pr = psum.tile([P, NC_], F32, name="plg", tag="plg")
                nc.tensor.matmul(pr, eff_rt[:], u[:, sl], start=True, stop=True)
                stg = stage_pool.tile([P, NC_], F32, name="stage", tag="stage")
                nc.scalar.activation(
                    stg, pr, AF.Identity, bias=br_sb[:, 0:1], scale=1.0
                )
                nc.gpsimd.dma_start(
                    out_ext.ap()[t * P : (t + 1) * P, sl], stg,
                    accum_op=ALU.add,
                )

    nc.compile()
    return nc


def _prep_shared(inputs):
    """Host-side prep of the replicated (weight) arrays."""
    sc = 0.125  # 1/sqrt(HD)

    def rep_logit(w):
        m = np.zeros((P, P), dtype=np.float32)
        ws = w.astype(np.float32) * sc
        m[:HD, :HD] = ws[:, None]          # rows d 0..63 -> head-0 columns
        m[HD:, HD:] = ws[:, None]          # rows d 64..127 -> head-1 columns
        return m.astype(BF)

    def bias_pp(b):
        return np.ascontiguousarray(b.astype(np.float32).reshape(T, P).T)

    wrr = np.zeros((P, P), dtype=np.float32)
    wr = inputs["Wr"].astype(np.float32)
    wrr[:HD, :HD] = wr
    wrr[HD:, HD:] = wr

    return {
        "wq": np.ascontiguousarray(inputs["Wq"].astype(BF)),
        "wk": np.ascontiguousarray(
            (inputs["Wk"].astype(np.float32) * W8SCALE).astype(F8)
        ),
        "wv": np.ascontiguousarray(
            (inputs["Wv"].astype(np.float32) * W8SCALE).astype(F8)
        ),
        "bq": bias_pp(inputs["bq"]),
        "bk": bias_pp(inputs["bk"]) * np.float32(W8SCALE),
        "bv": bias_pp(inputs["bv"]) * np.float32(W8SCALE),
        "wqlrep": rep_logit(inputs["wql"]),
        "wklrep": rep_logit(inputs["wkl"]),
        "wrr": wrr.astype(BF),
        "br": np.ascontiguousarray(
            np.tile(inputs["br"].astype(np.float32), 2).reshape(P, 1)
        ),
    }


def _get_nc():
    if "nc" not in _CACHE:
        _CACHE["nc"] = _build()
    return _CACHE["nc"]


def _run(inputs, trace=False):
    nc = _get_nc()
    shared = _prep_shared(inputs)
    X = inputs["X"]
    in_maps = []
    for b in range(N_CORES):
        m = dict(shared)
        xtb = np.ascontiguousarray(X[b].T)
        m["xt"] = xtb.astype(BF)
        m["xt8"] = xtb.astype(F8)
        in_maps.append(m)
    if trace:
        _install_profile_hook()
    res = run_bass_kernel_spmd(nc, in_maps, list(range(N_CORES)), trace=trace)
    out = np.empty((B, S, D), dtype=np.float32)
    for b in range(N_CORES):
        out[b] = res.results[b]["out"].T
    return out, res


def _install_profile_hook():
    import antenv

    if "antenv.axon_hooks" not in sys.modules:
        mod = types.ModuleType("antenv.axon_hooks")
        mod._hook = None
        mod.set_axon_ntff_profile_hook = lambda h: setattr(mod, "_hook", h)
        mod.get_axon_ntff_profile_hook = lambda: mod._hook
        sys.modules["antenv.axon_hooks"] = mod
        antenv.axon_hooks = mod
    hooks = sys.modules["antenv.axon_hooks"]
    if hooks.get_axon_ntff_profile_hook() is None:
        from trn_agent_boot.trn_boot import _ntff_profile_via_ctypes

        hooks.set_axon_ntff_profile_hook(
            _ntff_profile_via_ctypes("/opt/axon/libaxon_pjrt.so")
        )
    import concourse.bass_utils as bass_utils

    bass_utils.upload_artifacts = lambda tmpdir: f"local:{tmpdir}"


def kernel(**inputs) -> np.ndarray:
    out, _ = _run(inputs, trace=False)
    return out


# revision 26
# speedup vs baseline: 1.1430x; 1.1430x over previous
"""Trainium2 Bass kernel for nn_AdditiveAttention (B=8, S=4096, D=1024, H=16).

Sharding: pure data-parallel over batch — 8 NeuronCores, one batch element
per core, weights replicated. No collectives.

Per-core layout: everything transposed (d on partitions, s on free).
  - Host feeds xt = X[b].T as bf16 [1024, 4096]; device returns out.T f32.
  - Q/K/V: out_t[d', s] = sum_d W[d, d'] * xt[d, s]  (lhsT = W tile, rhs = xt)
  - per-head logits via block-diag "replicated" weight matmul: stationary
    [128, 128] where column j holds head(j//64)'s logit weights — every
    partition of the PSUM output carries its own head's logit row.
  - softmax over s without max-subtraction (|logits| << 1 by construction),
    exp + denominator fused on ScalarE via accum_out.
  - pooled vectors gq/gk via tensor_tensor mult + reduce_sum on VectorE.
  - p = (K + bk) * gq and u = (V + bv) * gk fused into PSUM evacuation
    (per-partition tensor_scalar).
  - residual: q + bq written straight to DRAM during Q evacuation;
    (u @ Wr + br) DMA-accumulated on top at the end.

Loop order inside each big matmul phase is (m, n, k): each PSUM chunk
accumulates its full contraction consecutively, so only ~2 banks are live
at a time and the logit/rt matmuls never starve for PSUM.
"""

import sys
import types

import numpy as np
import ml_dtypes

from contextlib import ExitStack

import concourse.bass as bass
import concourse.tile as tile
from concourse import bacc, mybir
from concourse.bass_utils import run_bass_kernel_spmd

B, S, D, H, HD = 8, 4096, 1024, 16, 64
P = 128          # partitions
T = D // P       # 8 d-tiles
NC_ = 512        # psum chunk free size
NS = S // NC_    # 8 s-chunks
N_CORES = 8
BF16 = mybir.dt.bfloat16
FP8 = mybir.dt.float8e4
F32 = mybir.dt.float32
W8SCALE = 64.0   # host scales Wk/Wv by this into e4m3 normal range
BF = ml_dtypes.bfloat16
F8 = ml_dtypes.float8_e4m3

_CACHE = {}


def _build():
    nc = bacc.Bacc(
        "TRN2", target_bir_lowering=False, debug=False, num_devices=N_CORES
    )
    xt_ext = nc.declare_dram_parameter("xt", [D, S], BF16, isOutput=False)
    wq_ext = nc.declare_dram_parameter("wq", [D, D], BF16, isOutput=False)
    xt8_ext = nc.declare_dram_parameter("xt8", [D, S], FP8, isOutput=False)
    wk_ext = nc.declare_dram_parameter("wk", [D, D], FP8, isOutput=False)
    wv_ext = nc.declare_dram_parameter("wv", [D, D], FP8, isOutput=False)
    bq_ext = nc.declare_dram_parameter("bq", [P, T], F32, isOutput=False)
    bk_ext = nc.declare_dram_parameter("bk", [P, T], F32, isOutput=False)
    bv_ext = nc.declare_dram_parameter("bv", [P, T], F32, isOutput=False)
    wql_ext = nc.declare_dram_parameter("wqlrep", [P, P], BF16, isOutput=False)
    wkl_ext = nc.declare_dram_parameter("wklrep", [P, P], BF16, isOutput=False)
    wrr_ext = nc.declare_dram_parameter("wrr", [P, P], BF16, isOutput=False)
    br_ext = nc.declare_dram_parameter("br", [P, 1], F32, isOutput=False)
    out_ext = nc.declare_dram_parameter("out", [D, S], F32, isOutput=True)

    AX = mybir.AxisListType.X
    ALU = mybir.AluOpType
    AF = mybir.ActivationFunctionType

    with tile.TileContext(nc) as tc, ExitStack() as ctx:
        singles = ctx.enter_context(tc.tile_pool(name="singles", bufs=1))
        wpool = ctx.enter_context(tc.tile_pool(name="wpool", bufs=2))
        psum = ctx.enter_context(tc.tile_pool(name="psum", bufs=4, space="PSUM"))
        qt_pool = ctx.enter_context(tc.tile_pool(name="qt", bufs=2))
        e_pool = ctx.enter_context(tc.tile_pool(name="epool", bufs=1))
        p_pool = ctx.enter_context(tc.tile_pool(name="ppool", bufs=3))
        u_pool = ctx.enter_context(tc.tile_pool(name="upool", bufs=2))
        stage_pool = ctx.enter_context(tc.tile_pool(name="stage", bufs=6))
        eff_pool = ctx.enter_context(tc.tile_pool(name="eff", bufs=2))
        small_pool = ctx.enter_context(tc.tile_pool(name="small", bufs=4))

        # ---- resident inputs ----
        # xt loaded s-major (one 3D-AP DMA per s-chunk: all 8 k-tiles) so the
        # first (m0, n0) contraction can start after ~1 MB instead of 8.4 MB,
        # and only 8 DMA issues are paid on the sync engine.
        # 16 DMAs (k-tile x s-half) with 4 KB-contiguous rows, spread over two
        # issuing engines so all 16 DMA engines pull concurrently.
        xt_sb = singles.tile([P, T, S], BF16, name="xt", tag="xt")
        for k in range(T):
            rsl = slice(k * P, (k + 1) * P)
            nc.sync.dma_start(
                xt_sb[:, k, : S // 2], xt_ext.ap()[rsl, : S // 2]
            )
            nc.gpsimd.dma_start(
                xt_sb[:, k, S // 2 :], xt_ext.ap()[rsl, S // 2 :]
            )
        wq_sb = wpool.tile([P, T, D], BF16, name="wmat", tag="wmat")
        for k in range(T):
            nc.scalar.dma_start(
                wq_sb[:, k, :], wq_ext.ap()[k * P : (k + 1) * P, :]
            )

        xt8_sb = singles.tile([P, T, S], FP8, name="xt8", tag="xt8")
        for k in range(T):
            nc.scalar.dma_start(
                xt8_sb[:, k, :], xt8_ext.ap()[k * P : (k + 1) * P, :]
            )

        def load_w(ext, dt):
            w_sb = wpool.tile([P, T, D], dt, name="wmat", tag="wmat")
            for k in range(T):
                nc.sync.dma_start(w_sb[:, k, :], ext.ap()[k * P : (k + 1) * P, :])
            return w_sb

        wqlrep = singles.tile([P, P], BF16, name="wqlrep", tag="wqlrep")
        nc.sync.dma_start(wqlrep[:], wql_ext.ap())
        wklrep = singles.tile([P, P], BF16, name="wklrep", tag="wklrep")
        nc.sync.dma_start(wklrep[:], wkl_ext.ap())
        wrr = singles.tile([P, P], BF16, name="wrr", tag="wrr")
        nc.sync.dma_start(wrr[:], wrr_ext.ap())
        bq_sb = singles.tile([P, T], F32, name="bq", tag="bq")
        nc.sync.dma_start(bq_sb[:], bq_ext.ap())
        bk_sb = singles.tile([P, T], F32, name="bk", tag="bk")
        nc.sync.dma_start(bk_sb[:], bk_ext.ap())
        bv_sb = singles.tile([P, T], F32, name="bv", tag="bv")
        nc.sync.dma_start(bv_sb[:], bv_ext.ap())
        br_sb = singles.tile([P, 1], F32, name="br", tag="br")
        nc.sync.dma_start(br_sb[:], br_ext.ap())

        gq_all = singles.tile([P, T], F32, name="gq", tag="gq")
        gk_all = singles.tile([P, T], F32, name="gk", tag="gk")

        def big_mm_chunk(w_sb, t, n):
            """One fully-accumulated [P, NC_] psum chunk of output tile t."""
            pch = psum.tile([P, NC_], F32, name="pch", tag="pch")
            for k in range(T):
                nc.tensor.matmul(
                    pch,
                    w_sb[:, k, t * P : (t + 1) * P],
                    xt_sb[:, k, n * NC_ : (n + 1) * NC_],
                    start=(k == 0),
                    stop=(k == T - 1),
                )
            return pch

        def big_mm_chunk_f8(w_sb, t, n):
            """fp8 DoubleRow version: k-tiles paired, 2 MACs/cell/cycle."""
            pch = psum.tile([P, NC_], F32, name="pch", tag="pch")
            for k in range(0, T, 2):
                nc.tensor.matmul(
                    pch,
                    w_sb[:, k : k + 2, t * P : (t + 1) * P],
                    xt8_sb[:, k : k + 2, n * NC_ : (n + 1) * NC_],
                    start=(k == 0),
                    stop=(k == T - 2),
                    perf_mode=mybir.MatmulPerfMode.DoubleRow,
                )
            return pch

        def pool_step(t, src_tile, wrep, g_all):
            """softmax(logits)-weighted sum of src_tile over s -> g_all[:, t]."""
            e = e_pool.tile([P, S], BF16, name="e", tag="e")
            sums = small_pool.tile([P, NS], F32, name="sums", tag="sums")
            for n in range(NS):
                pc = psum.tile([P, NC_], F32, name="plg", tag="plg")
                nc.tensor.matmul(
                    pc, wrep, src_tile[:, n * NC_ : (n + 1) * NC_],
                    start=True, stop=True,
                )
                nc.scalar.activation(
                    e[:, n * NC_ : (n + 1) * NC_], pc, AF.Exp,
                    bias=0.0, scale=1.0, accum_out=sums[:, n : n + 1],
                )
            stot = small_pool.tile([P, 1], F32, name="stot", tag="stot")
            nc.vector.reduce_sum(stot, sums, axis=AX)
            # fold the 1/W8SCALE un-scaling of the fp8 K/V matmuls into the
            # softmax normalizer: g_all ends up holding g / W8SCALE.
            nc.vector.tensor_scalar_mul(stot, stot, W8SCALE)
            recip = small_pool.tile([P, 1], F32, name="recip", tag="recip")
            nc.vector.reciprocal(recip, stot)
            graw = small_pool.tile([P, 1], F32, name="graw", tag="graw")
            nc.vector.tensor_tensor(e[:], e[:], src_tile[:], ALU.mult)
            nc.vector.reduce_sum(graw, e[:], axis=AX)
            nc.vector.tensor_mul(g_all[:, t : t + 1], graw, recip)

        # ---- Q phase ----
        for t in range(T):
            qt = qt_pool.tile([P, S], BF16, name="qt", tag="qt")
            for n in range(NS):
                sl = slice(n * NC_, (n + 1) * NC_)
                pch = big_mm_chunk(wq_sb, t, n)
                stg = stage_pool.tile([P, NC_], F32, name="stage", tag="stage")
                nc.scalar.activation(
                    stg, pch, AF.Identity, bias=bq_sb[:, t : t + 1], scale=1.0
                )
                nc.sync.dma_start(out_ext.ap()[t * P : (t + 1) * P, sl], stg)
                nc.vector.tensor_copy(qt[:, sl], stg)
            pool_step(t, qt, wqlrep, gq_all)

        # ---- K phase ----
        wk_sb = load_w(wk_ext, FP8)
        for t in range(T):
            # pk = 64*(k + bk), ungated: the gq gate (and the 1/64) ride in
            # the kl stationary and the gk finalizer instead.
            p = p_pool.tile([P, S], BF16, name="p", tag="p")
            for n in range(NS):
                pch = big_mm_chunk_f8(wk_sb, t, n)
                nc.scalar.activation(
                    p[:, n * NC_ : (n + 1) * NC_], pch, AF.Identity,
                    bias=bk_sb[:, t : t + 1], scale=1.0,
                )
            eff_kl = eff_pool.tile([P, P], BF16, name="effkl", tag="effkl")
            nc.vector.tensor_scalar_mul(eff_kl[:], wklrep[:], gq_all[:, t : t + 1])
            pool_step(t, p, eff_kl, gk_all)
            nc.vector.tensor_mul(
                gk_all[:, t : t + 1], gk_all[:, t : t + 1], gq_all[:, t : t + 1]
            )

        # ---- V phase + output (chunk-granular: u, rt, evac, DMA per n) ----
        wv_sb = load_w(wv_ext, FP8)
        for t in range(T):
            u = u_pool.tile([P, S], BF16, name="u", tag="u")
            eff_rt = eff_pool.tile([P, P], BF16, name="effrt", tag="effrt")
            nc.vector.tensor_scalar_mul(eff_rt[:], wrr[:], gk_all[:, t : t + 1])
            for n in range(NS):
                sl = slice(n * NC_, (n + 1) * NC_)
                pch = big_mm_chunk_f8(wv_sb, t, n)
                nc.scalar.activation(
                    u[:, sl], pch, AF.Identity,
                    bias=bv_sb[:, t : t + 1], scale=1.0,
                )
                pr = psum.tile([P, NC_], F32, name="plg", tag="plg")
                nc.tensor.matmul(pr, eff_rt[:], u[:, sl], start=True, stop=True)
                stg = stage_pool.tile([P, NC_], F32, name="stage", tag="stage")
                nc.scalar.activation(
                    stg, pr, AF.Identity, bias=br_sb[:, 0:1], scale=1.0
                )
                nc.gpsimd.dma_start(
                    out_ext.ap()[t * P : (t + 1) * P, sl], stg,
                    accum_op=ALU.add,
                )

    nc.compile()
    return nc


def _prep_shared(inputs):
    """Host-side prep of the replicated (weight) arrays."""
    sc = 0.125  # 1/sqrt(HD)

    def rep_logit(w):
        m = np.zeros((P, P), dtype=np.float32)
        ws = w.astype(np.float32) * sc
        m[:HD, :HD] = ws[:, None]          # rows d 0..63 -> head-0 columns
        m[HD:, HD:] = ws[:, None]          # rows d 64..127 -> head-1 columns
        return m.astype(BF)

    def bias_pp(b):
        return np.ascontiguousarray(b.astype(np.float32).reshape(T, P).T)

    wrr = np.zeros((P, P), dtype=np.float32)
    wr = inputs["Wr"].astype(np.float32)
    wrr[:HD, :HD] = wr
    wrr[HD:, HD:] = wr

    return {
        "wq": np.ascontiguousarray(inputs["Wq"].astype(BF)),
        "wk": np.ascontiguousarray(
            (inputs["Wk"].astype(np.float32) * W8SCALE).astype(F8)
        ),
        "wv": np.ascontiguousarray(
            (inputs["Wv"].astype(np.float32) * W8SCALE).astype(F8)
        ),
        "bq": bias_pp(inputs["bq"]),
        "bk": bias_pp(inputs["bk"]) * np.float32(W8SCALE),
        "bv": bias_pp(inputs["bv"]) * np.float32(W8SCALE),
        "wqlrep": rep_logit(inputs["wql"]),
        "wklrep": rep_logit(inputs["wkl"]),
        "wrr": wrr.astype(BF),
        "br": np.ascontiguousarray(
            np.tile(inputs["br"].astype(np.float32), 2).reshape(P, 1)
        ),
    }


def _get_nc():
    if "nc" not in _CACHE:
        _CACHE["nc"] = _build()
    return _CACHE["nc"]


def _run(inputs, trace=False):
    nc = _get_nc()
    shared = _prep_shared(inputs)
    X = inputs["X"]
    in_maps = []
    for b in range(N_CORES):
        m = dict(shared)
        xtb = np.ascontiguousarray(X[b].T)
        m["xt"] = xtb.astype(BF)
        m["xt8"] = xtb.astype(F8)
        in_maps.append(m)
    if trace:
        _install_profile_hook()
    res = run_bass_kernel_spmd(nc, in_maps, list(range(N_CORES)), trace=trace)
    out = np.empty((B, S, D), dtype=np.float32)
    for b in range(N_CORES):
        out[b] = res.results[b]["out"].T
    return out, res


def _install_profile_hook():
    import antenv

    if "antenv.axon_hooks" not in sys.modules:
        mod = types.ModuleType("antenv.axon_hooks")
        mod._hook = None
        mod.set_axon_ntff_profile_hook = lambda h: setattr(mod, "_hook", h)
        mod.get_axon_ntff_profile_hook = lambda: mod._hook
        sys.modules["antenv.axon_hooks"] = mod
        antenv.axon_hooks = mod
    hooks = sys.modules["antenv.axon_hooks"]
    if hooks.get_axon_ntff_profile_hook() is None:
        from trn_agent_boot.trn_boot import _ntff_profile_via_ctypes

        hooks.set_axon_ntff_profile_hook(
            _ntff_profile_via_ctypes("/opt/axon/libaxon_pjrt.so")
        )
    import concourse.bass_utils as bass_utils

    bass_utils.upload_artifacts = lambda tmpdir: f"local:{tmpdir}"


def kernel(**inputs) -> np.ndarray:
    out, _ = _run(inputs, trace=False)
    return out
